# revision 8
# baseline (speedup 1.0000x reference)
"""Inverse Haar DWT2 (pywt 'haar' idwt2 convention) on 8 Trainium2 cores.

Input  x: [16, 256, 128, 128] f32 — 4 stacked subbands (LL|LH|HL|HH) of 64
channels each.  Output: [16, 64, 256, 256] f32.

Sharding: batch dim (16) split across 8 cores, 2 batches per core.  The
transform is elementwise per (batch, channel) — no communication.

Per-core kernel (x_loc [2, 256, 128, 128] -> y_loc [2, 64, 256, 256]):
SBUF partition dim = (channel, batch) = 64*2 = 128; free dim = a chunk of
HC input rows * 128 cols.  Per iteration (16 of them, HC=8):
  - ONE load DMA on the SP HWDGE ring: T [128p, band*HC*128]; DRAM inner
    run is HC*512B = 4KB contiguous, outer source dim 64 for engine spray
  - stage 1 on GpSimd (Pool): U0|U1 = LL +- LH, V0|V1 = HL +- HH
    (plain tensor_tensor; walrus rejects TensorScalarPtr on Pool)
  - V0h|V1h = V0|V1 * 0.5 on ScalarE (ACT)
  - stage 2 on DVE (scalar_tensor_tensor, remaining *0.5 folded in):
    out[2i+r, 2j+s] = U_r*0.5 +- V_rh, stride-2 column interleave into
    OUT [128p, i*2*2W + r*2W + 2j+s]
  - ONE store DMA on the ACT HWDGE ring (separate ring from the loads, so
    a store waiting on compute never head-of-line-blocks the next load);
    output rows consecutive per (c,b): DRAM inner run 2*HC*256*4B = 16KB
All pools are at least double-buffered so loads prefetch ahead and the
per-iter compute chain (ACT 0.9us -> Pool 3us -> DVE 4.3us) hides under
the ~10.5us/iter of DMA.  HBM traffic per core = 32 MiB in + 32 MiB out
-> ~187 us roofline at ~358 GB/s per-NC HBM bandwidth.

This container's walrus build supports only ONE semaphore wait per
instruction; Tile emits multi-wait instructions (incl. the final drain), so
after TileContext exit we redistribute extra waits onto single-wait NOPs
inserted before the instruction on the same engine.
"""

import numpy as np

import concourse.bass as bass
import concourse.mybir as mybir
from concourse.tile import TileContext
from concourse.bass_utils import run_bass_kernel_spmd

N_CORES = 8
B, C4, H, W = 16, 256, 128, 128
CH = C4 // 4          # 64 output channels
B_LOC = B // N_CORES  # 2 batches per core
HC = 8                # input rows per tile iteration
F32 = mybir.dt.float32
ALU = mybir.AluOpType


def _split_multi_waits(nc):
    """Move extra semaphore waits onto single-wait NOPs placed immediately
    before the over-subscribed instruction (same engine, so per-engine
    program order is preserved)."""
    n_split = 0
    for f in nc.m.functions:
        for blk in f.blocks:
            il = blk.instructions
            new_list = []
            for inst in il:
                si = getattr(inst, "sync_info", None)
                ow = si.on_wait if si is not None else None
                if ow and len(ow) > 1:
                    extra = list(ow[:-1])
                    del ow[:-1]
                    for w in extra:
                        n_split += 1
                        new_list.append(
                            mybir.InstNoOp(
                                name=f"{inst.name}-waitsplit-{n_split}",
                                engine=inst.engine,
                                sync_info=mybir.SyncInfo(on_wait=[w], on_update=[]),
                            )
                        )
                new_list.append(inst)
            il[:] = new_list
    return n_split


def _build_kernel(h=H, hc=HC, split_waits=True):
    nc = bass.Bass("TRN2")
    x = nc.dram_tensor("x", [B_LOC, C4, h, W], F32, kind="ExternalInput")
    y = nc.dram_tensor("y", [B_LOC, CH, 2 * h, 2 * W], F32, kind="ExternalOutput")

    FB = hc * W          # free elems per band block
    with TileContext(nc) as tc:
        with (
            tc.tile_pool(name="tin", bufs=3) as pin,
            tc.tile_pool(name="ts", bufs=2) as ps,
            tc.tile_pool(name="tw", bufs=2) as pw,
            tc.tile_pool(name="tout", bufs=3) as pout,
        ):
            for it in range(h // hc):
                h0 = it * hc
                # ---- load: one DMA, T [p=(c,b)][band][i][w]
                # partition p = c*2 + b so the DRAM AP's outermost dim has
                # count 64 (the HWDGE engine spray follows the outer source
                # dim; outer count 2 would use only 2 of 16 SDMA engines)
                T = pin.tile([128, 4 * FB], F32, tag="T")
                nc.sync.dma_start(
                    out=T[:].rearrange("p (band x) -> p band x", band=4),
                    in_=x[:, :, h0 : h0 + hc, :]
                    .rearrange("b (band c) h w -> c b band (h w)", band=4),
                )
                Tb = T[:].rearrange("p (band x) -> p band x", band=4)
                # ---- stage 1: vertical butterfly (GpSimd / Pool, plain
                # add/sub only — walrus rejects TensorScalarPtr on Pool)
                # U0|U1 = LL +- LH ; V0|V1 = HL +- HH
                WK = pw.tile([128, 4 * FB], F32, tag="WK")
                Wb = WK[:].rearrange("p (k x) -> p k x", k=4)
                nc.gpsimd.tensor_add(out=Wb[:, 0], in0=Tb[:, 0], in1=Tb[:, 1])
                nc.gpsimd.tensor_sub(out=Wb[:, 1], in0=Tb[:, 0], in1=Tb[:, 1])
                nc.gpsimd.tensor_add(out=Wb[:, 2], in0=Tb[:, 2], in1=Tb[:, 3])
                nc.gpsimd.tensor_sub(out=Wb[:, 3], in0=Tb[:, 2], in1=Tb[:, 3])
                # ---- V0h|V1h = V0|V1 * 0.5 (ScalarE ACT; lets stage 2
                # fold its 0.5 onto the U operand via scalar_tensor_tensor)
                S = ps.tile([128, 2 * FB], F32, tag="S")
                nc.scalar.mul(S[:], WK[:, 2 * FB :], 0.5)
                # ---- stage 2 (DVE): horizontal butterfly + column
                # interleave, *0.5 folded onto the U operand.
                # OUT free layout [i][r][col], col = 2j+s.  Keep every AP at
                # <=2 free dims — 3-free-dim strided DVE ops run ~2x slower.
                OUT = pout.tile([128, 2 * hc * 2 * W], F32, tag="OUT")
                OUTv = OUT[:].rearrange(
                    "p (i r j s) -> p i r j s", i=hc, r=2, j=W, s=2
                )
                Wv = WK[:].rearrange("p (k i w) -> p k i w", k=4, i=hc)
                Sv = S[:].rearrange("p (k i w) -> p k i w", k=2, i=hc)
                for r in range(2):
                    u = Wv[:, r]
                    v = Sv[:, r]
                    nc.vector.scalar_tensor_tensor(
                        out=OUTv[:, :, r, :, 0], in0=u, scalar=0.5, in1=v,
                        op0=ALU.mult, op1=ALU.add,
                    )
                    nc.vector.scalar_tensor_tensor(
                        out=OUTv[:, :, r, :, 1], in0=u, scalar=0.5, in1=v,
                        op0=ALU.mult, op1=ALU.subtract,
                    )
                # ---- store rows 2*h0 .. 2*h0+2*hc-1 (consecutive) on the
                # ACT HWDGE ring
                nc.scalar.dma_start(
                    out=y[:, :, 2 * h0 : 2 * h0 + 2 * hc, :]
                    .rearrange("b c h w -> c b (h w)"),
                    in_=OUT[:],
                )

    if split_waits:
        _split_multi_waits(nc)
    return nc


_NC_CACHE = None


def _get_nc():
    global _NC_CACHE
    if _NC_CACHE is None:
        _NC_CACHE = _build_kernel()
    return _NC_CACHE


def run_sharded(x, trace=False, **kwargs):
    assert x.shape == (B, C4, H, W) and x.dtype == np.float32
    nc = _get_nc()
    in_maps = [
        {"x": np.ascontiguousarray(x[i * B_LOC : (i + 1) * B_LOC])}
        for i in range(N_CORES)
    ]
    res = run_bass_kernel_spmd(
        nc, in_maps, core_ids=list(range(N_CORES)), trace=trace, **kwargs
    )
    out = np.concatenate([r["y"] for r in res.results], axis=0)
    return out, res


def kernel(x):
    out, _ = run_sharded(np.asarray(x))
    return out
